# revision 11
# baseline (speedup 1.0000x reference)
"""Trainium2 Bass kernel for the Net_SDE neural-SDE Monte Carlo problem.

Computes 48 Euler steps of a neural SDE over 131072 MC paths, where drift
and diffusion of (S, V) come from four small MLPs (3->100->100->100->100->1),
and accumulates per-strike discounted payoff means at each step.

Strategy (8 NeuronCores, data-parallel over MC paths, 16384 paths/core):
  - Feature-major matmuls: activations stored [features(part), samples(free)],
    weights stationary, samples stream through the PE.  float32r matmuls
    (1 cyc/row vs 4 for fp32).
  - Bias+ReLU fused into the PSUM->SBUF evacuation op (split ACT/DVE).
  - Final (100->1) layers of the four nets packed into one PSUM [4, 512]
    accumulation group via zero-padded w5 columns.
  - SDE state update + 40 strike payoff partial sums on [128,128] "brick"
    tiles (sample j lives at brick[j // 128, j % 128]); per-step per-partition
    partial sums are DMA'd to DRAM; host reduces over cores+partitions,
    discounts, and selects the 24 observation steps.
"""

import math
import os

import numpy as np

N_STEPS = 48
MC = 131072
N_CORES = 8
MCL = MC // N_CORES        # 16384 paths per core
BC = 128                   # brick columns; MCL == 128 * BC
WIDTH = 100
NT = MCL // 1024           # 16 sample-tiles of 1024 per step
H = 2.0 / N_STEPS
SQRTH = math.sqrt(H)
TIMEGRID = np.linspace(0.0, 2.0, N_STEPS + 1)
STRIKES_CALL = [100.0, 105.0, 110.0, 115.0, 120.0, 125.0, 130.0, 135.0, 140.0, 145.0]
STRIKES_PUT = [55.0, 60.0, 65.0, 70.0, 75.0, 80.0, 85.0, 90.0, 95.0, 100.0]
NETS = ("diffusion", "driftV", "diffusionV", "diffusionV1")

# fraction of hidden-layer PSUM evacuation ops assigned to the ACT engine
ACT_EVAC_NUM = 35
ACT_EVAC_DEN = 64

_PROGRAM_CACHE = {}


def _build_program(n_steps=N_STEPS, mm_mode="f32r"):
    import concourse.bass as bass
    import concourse.bacc as bacc
    import concourse.mybir as mybir
    import concourse.tile as tile
    from contextlib import ExitStack

    f32 = mybir.dt.float32
    f32r = mybir.dt.float32r
    Relu = mybir.ActivationFunctionType.Relu
    add = mybir.AluOpType.add
    mult = mybir.AluOpType.mult
    amax = mybir.AluOpType.max
    ds = bass.ds

    MDT = f32r if mm_mode == "f32r" else f32

    assert n_steps % 2 == 0

    nc = bacc.Bacc("TRN2", target_bir_lowering=False, debug=False,
                   num_devices=N_CORES)

    # ---- DRAM I/O ----
    zs_d = nc.dram_tensor("zs", [n_steps, MCL], f32, kind="ExternalInput").ap()
    z1s_d = nc.dram_tensor("z1s", [n_steps, MCL], f32, kind="ExternalInput").ap()
    w1s_d = nc.dram_tensor("w1s", [2, 400], MDT, kind="ExternalInput").ap()
    b1e_d = nc.dram_tensor("b1e", [WIDTH, 4 * n_steps], f32, kind="ExternalInput").ap()
    wh_d = nc.dram_tensor("wh", [WIDTH, 1200], MDT, kind="ExternalInput").ap()
    bh_d = nc.dram_tensor("bh", [WIDTH, 12], f32, kind="ExternalInput").ap()
    w5z_d = nc.dram_tensor("w5z", [WIDTH, 16], MDT, kind="ExternalInput").ap()
    cns_d = nc.dram_tensor("cns", [128, 28], f32, kind="ExternalInput").ap()
    x0_d = nc.dram_tensor("x0", [2, MCL], MDT, kind="ExternalInput").ap()
    acc_d = nc.dram_tensor("acc", [n_steps * 128, 40], f32,
                           kind="ExternalOutput").ap()

    with tile.TileContext(nc) as tc, ExitStack() as ctx:
        wp = ctx.enter_context(tc.tile_pool(name="wp", bufs=1))
        sp = ctx.enter_context(tc.tile_pool(name="sp", bufs=1))
        ap_pool = ctx.enter_context(tc.tile_pool(name="ap", bufs=8))
        bt_pool = ctx.enter_context(tc.tile_pool(name="bt", bufs=4))
        ph = ctx.enter_context(tc.tile_pool(name="ph", bufs=3, space="PSUM"))
        p5p = ctx.enter_context(tc.tile_pool(name="p5", bufs=2, space="PSUM"))

        # ---- persistent SBUF tiles ----
        w1s = wp.tile([2, 400], MDT, tag="w1s")
        b1e = wp.tile([WIDTH, 4 * n_steps], f32, tag="b1e")
        wh = wp.tile([WIDTH, 1200], MDT, tag="wh")
        bh = wp.tile([WIDTH, 12], f32, tag="bh")
        w5z = wp.tile([WIDTH, 16], MDT, tag="w5z")
        cns = wp.tile([128, 28], f32, tag="cns")
        x_sb = wp.tile([2, MCL], MDT, tag="x_sb")
        zeros = wp.tile([128, 128], f32, tag="zeros")
        o_rows = wp.tile([4, MCL], f32, tag="o_rows")
        ob = [sp.tile([128, BC], f32, tag=f"ob{n}", name=f"ob{n}") for n in range(4)]
        scr_d = sp.tile([128, 128], f32, tag="scr_d")
        scr_a = sp.tile([128, 128], f32, tag="scr_a")
        sbr = [sp.tile([128, BC], MDT, tag=f"sbr{p}", name=f"sbr{p}") for p in range(2)]
        vbr = [sp.tile([128, BC], MDT, tag=f"vbr{p}", name=f"vbr{p}") for p in range(2)]
        zb = [sp.tile([128, BC], f32, tag=f"zb{p}", name=f"zb{p}") for p in range(2)]
        z1b = [sp.tile([128, BC], f32, tag=f"z1b{p}", name=f"z1b{p}") for p in range(2)]
        acc_t = [sp.tile([128, 40], f32, tag=f"acc{p}", name=f"acc{p}") for p in range(2)]

        # ---- preload ----
        nc.sync.dma_start(w1s[:], w1s_d)
        nc.sync.dma_start(b1e[:], b1e_d)
        nc.sync.dma_start(wh[:], wh_d)
        nc.sync.dma_start(bh[:], bh_d)
        nc.sync.dma_start(w5z[:], w5z_d)
        nc.sync.dma_start(cns[:], cns_d)
        nc.sync.dma_start(x_sb[:], x0_d)
        nc.sync.dma_start(sbr[1][:], x0_d[0:1, :])
        nc.sync.dma_start(vbr[1][:], x0_d[1:2, :])
        nc.vector.memset(zeros[:], 0.0)

        evac_ctr = [0]

        def evac(out_ap, in_ap, bias_ap):
            k = evac_ctr[0]
            evac_ctr[0] += 1
            if (k * ACT_EVAC_NUM) % ACT_EVAC_DEN < ACT_EVAC_NUM:
                nc.scalar.activation(out_ap, in_ap, Relu, bias=bias_ap, scale=1.0)
            else:
                nc.vector.tensor_scalar(out_ap, in_ap, bias_ap, 0.0, add, amax)

        def emit_step(step, par):
            so, vo = sbr[1 - par], vbr[1 - par]   # state from previous step
            sn, vn = sbr[par], vbr[par]           # this step's new state

            # z bricks for this step
            nc.sync.dma_start(zb[par][:], zs_d[ds(step, 1), :])
            nc.sync.dma_start(z1b[par][:], z1s_d[ds(step, 1), :])

            for t in range(NT):
                a_prev = [None] * 4
                # L1: K=2 (S,V rows); bias (incl. t*W1[0]+b1) fused in evac
                for n in range(4):
                    ps = ph.tile([WIDTH, 1024], f32, tag="ps", name="ps")
                    for k in range(2):
                        nc.tensor.matmul(
                            ps[:, k * 512:(k + 1) * 512],
                            w1s[:, n * 100:(n + 1) * 100],
                            x_sb[:, t * 1024 + k * 512: t * 1024 + (k + 1) * 512],
                            start=True, stop=True)
                    a1 = ap_pool.tile([WIDTH, 1024], MDT, tag="a", name="a1")
                    evac(a1[:], ps[:], b1e[:, 4 * step + n: 4 * step + n + 1])
                    a_prev[n] = a1
                # L2..L4 hidden layers
                for l in range(3):
                    for n in range(4):
                        ps = ph.tile([WIDTH, 1024], f32, tag="ps", name="ps")
                        for k in range(2):
                            nc.tensor.matmul(
                                ps[:, k * 512:(k + 1) * 512],
                                wh[:, n * 300 + l * 100: n * 300 + (l + 1) * 100],
                                a_prev[n][:, k * 512:(k + 1) * 512],
                                start=True, stop=True)
                        a2 = ap_pool.tile([WIDTH, 1024], MDT, tag="a", name="a2")
                        evac(a2[:], ps[:], bh[:, n * 3 + l: n * 3 + l + 1])
                        a_prev[n] = a2
                # L5: four nets into one PSUM [4,512] via zero-padded w5 cols
                for k in range(2):
                    p5t = p5p.tile([4, 512], f32, tag="p5", name="p5t")
                    for n in range(4):
                        nc.tensor.matmul(
                            p5t[:],
                            w5z[:, n * 4:(n + 1) * 4],
                            a_prev[n][:, k * 512:(k + 1) * 512],
                            start=(n == 0), stop=(n == 3))
                    nc.vector.tensor_copy(
                        o_rows[:, t * 1024 + k * 512: t * 1024 + (k + 1) * 512],
                        p5t[:])

            # rows -> bricks (per net) for the state update
            for n in range(4):
                nc.sync.dma_start(ob[n][:], o_rows[n:n + 1, :])

            o_d, o_dr, o_dv, o_dv1 = ob[0], ob[1], ob[2], ob[3]

            def btile():
                return bt_pool.tile([128, 128], f32, tag="btmp", name="btmp")

            # S update: S' = relu(c1*S + relu(o_d + b5d)*dW)
            t1 = btile()
            nc.vector.scalar_tensor_tensor(t1[:], o_d[:], cns[:, 1:2], zeros[:],
                                           add, amax)
            sdw = btile()
            nc.vector.tensor_tensor(sdw[:], t1[:], zb[par][:], mult)
            s1 = btile()
            nc.vector.scalar_tensor_tensor(s1[:], so[:], cns[:, 0:1], sdw[:],
                                           mult, add)
            nc.vector.tensor_scalar_max(sn[:], s1[:], 0.0)

            # V update: V' = relu(V + (o_dr+b5dr)*h + relu(o_dv+b5dv)*dW
            #                      + relu(o_dv1+b5dv1)*dW1)
            t2 = btile()
            nc.vector.scalar_tensor_tensor(t2[:], o_dv[:], cns[:, 3:4], zeros[:],
                                           add, amax)
            a2b = btile()
            nc.vector.tensor_tensor(a2b[:], t2[:], zb[par][:], mult)
            t3 = btile()
            nc.vector.scalar_tensor_tensor(t3[:], o_dv1[:], cns[:, 4:5], zeros[:],
                                           add, amax)
            a3b = btile()
            nc.vector.tensor_tensor(a3b[:], t3[:], z1b[par][:], mult)
            u1 = btile()
            nc.vector.tensor_scalar(u1[:], o_dr[:], cns[:, 2:3], float(H),
                                    add, mult)
            v1 = btile()
            nc.vector.tensor_tensor(v1[:], u1[:], vo[:], add)
            v2 = btile()
            nc.vector.tensor_tensor(v2[:], v1[:], a2b[:], add)
            v3 = btile()
            nc.vector.tensor_tensor(v3[:], v2[:], a3b[:], add)
            nc.vector.tensor_scalar_max(vn[:], v3[:], 0.0)

            # payoff partial sums (per partition) for all 40 strikes
            at = acc_t[par]
            X = mybir.AxisListType.X
            for j, K in enumerate(STRIKES_CALL):   # call OTM
                nc.vector.tensor_scalar(scr_d[:], sn[:], -float(K), 0.0, add, amax)
                nc.vector.tensor_reduce(at[:, j:j + 1], scr_d[:], X, add)
            for j, K in enumerate(STRIKES_PUT):    # call ITM
                nc.vector.tensor_scalar(scr_d[:], sn[:], -float(K), 0.0, add, amax)
                nc.vector.tensor_reduce(at[:, 20 + j:21 + j], scr_d[:], X, add)
            for j in range(10):                    # put OTM (bias=K_put)
                nc.scalar.activation(scr_a[:], sn[:], Relu, bias=cns[:, 8 + j:9 + j],
                                     scale=-1.0, accum_out=at[:, 10 + j:11 + j])
            for j in range(10):                    # put ITM (bias=K_call)
                nc.scalar.activation(scr_a[:], sn[:], Relu, bias=cns[:, 18 + j:19 + j],
                                     scale=-1.0, accum_out=at[:, 30 + j:31 + j])
            nc.sync.dma_start(acc_d[ds(step * 128, 128), :], at[:])

            # write next step's x rows
            nc.sync.dma_start(x_sb[0:1, :], sn[:])
            nc.sync.dma_start(x_sb[1:2, :], vn[:])

        for k in range(n_steps):
            emit_step(k, k % 2)

    nc.compile()
    return nc


def _prep_inputs(S0, V0, rate, z, z1, params, n_steps=N_STEPS):
    """Host-side packing.  Returns per-core input maps."""
    f32 = np.float32
    S0 = float(np.asarray(S0).reshape(-1)[0])
    V0 = float(np.asarray(V0).reshape(-1)[0])
    r = float(np.asarray(rate).reshape(-1)[0])
    z = np.asarray(z, dtype=f32)
    z1 = np.asarray(z1, dtype=f32)

    def layers(name):
        return [(np.asarray(W, dtype=f32), np.asarray(b, dtype=f32))
                for (W, b) in params[name]]

    nets = [layers(n) for n in NETS]

    w1s = np.zeros((2, 400), f32)
    b1e = np.zeros((WIDTH, 4 * n_steps), f32)
    wh = np.zeros((WIDTH, 1200), f32)
    bh = np.zeros((WIDTH, 12), f32)
    w5z = np.zeros((WIDTH, 16), f32)
    cns = np.zeros((128, 28), f32)
    cns[:, 0] = f32(1.0) + f32(r) * f32(H)
    cns[:, 8:18] = np.asarray(STRIKES_PUT, f32)
    cns[:, 18:28] = np.asarray(STRIKES_CALL, f32)
    for n, L in enumerate(nets):
        W1, b1 = L[0]
        w1s[:, n * 100:(n + 1) * 100] = W1[1:3, :]
        for k in range(n_steps):
            b1e[:, 4 * k + n] = b1 + f32(TIMEGRID[k]) * W1[0, :]
        for l in range(3):
            Wl, bl = L[1 + l]
            wh[:, n * 300 + l * 100: n * 300 + (l + 1) * 100] = Wl
            bh[:, n * 3 + l] = bl
        W5, b5 = L[4]
        w5z[:, n * 4 + n] = W5[:, 0]
        cns[:, 1 + n] = b5[0]

    x0 = np.zeros((2, MCL), f32)
    x0[0, :] = S0
    x0[1, :] = V0

    shared = dict(w1s=w1s, b1e=b1e, wh=wh, bh=bh, w5z=w5z, cns=cns, x0=x0)

    per_core = []
    for c in range(N_CORES):
        zc = z[c * MCL:(c + 1) * MCL, :n_steps]
        z1c = z1[c * MCL:(c + 1) * MCL, :n_steps]
        zs = np.ascontiguousarray((zc.T * f32(SQRTH)).astype(f32))
        z1s = np.ascontiguousarray((z1c.T * f32(SQRTH)).astype(f32))
        per_core.append(dict(shared, zs=zs, z1s=z1s))
    return per_core


def _postprocess(acc_list, rate, indices, n_steps=N_STEPS):
    r = float(np.asarray(rate).reshape(-1)[0])
    tot = np.zeros((n_steps, 40), np.float64)
    for a in acc_list:
        tot += a.astype(np.float64).reshape(n_steps, 128, 40).sum(axis=1)
    means = tot / MC
    i_f = np.arange(1, n_steps + 1, dtype=np.float64)
    disc = np.exp(-r * 2.0 * i_f / N_STEPS)
    prices = means * disc[:, None]           # [n_steps, 40]
    prices = prices.reshape(n_steps, 4, 10)  # [call_otm, put_otm, call_itm, put_itm]
    idx = np.asarray(indices).astype(np.int64).reshape(-1)
    sel = prices[idx - 1]                    # negative indices wrap like jnp
    out = np.transpose(sel, (1, 0, 2)).reshape(4 * idx.shape[0], 10)
    return out.astype(np.float32)


def _get_program(n_steps=N_STEPS, mm_mode=None):
    if mm_mode is None:
        mm_mode = os.environ.get("SDE_MM_MODE", "f32r")
    key = (n_steps, mm_mode)
    if key not in _PROGRAM_CACHE:
        _PROGRAM_CACHE[key] = _build_program(n_steps, mm_mode)
    return _PROGRAM_CACHE[key]


def kernel(S0, V0, rate, indices, z, z1, params, **_ignored):
    from concourse.bass_utils import run_bass_kernel_spmd

    nc = _get_program()
    per_core = _prep_inputs(S0, V0, rate, z, z1, params)
    core_ids = list(range(N_CORES))
    trace = bool(int(os.environ.get("SDE_TRACE", "0")))
    res = run_bass_kernel_spmd(nc, per_core, core_ids, trace=trace)
    kernel.last_results = res
    acc_list = [res.results[i]["acc"] for i in range(N_CORES)]
    return _postprocess(acc_list, rate, indices)
